# revision 12
# baseline (speedup 1.0000x reference)
"""Trainium2 Bass/Tile kernel for nn_AccumAtt (temporal accumulated attention).

Pipeline (per (b, t) frame of x [B*T, C, H, W]):
  xv = spatial mean -> left/right = relu(BN(xv @ w{1,2}.T)) -> temporal diff
  -> sequential gate scan over T -> att = sigmoid(new @ Wa.T) -> out = x * att.

Sharding: data-parallel over batch. 8 cores x 2 batch elements each; params
replicated. Single kernel streams each frame once: load -> reduce -> tiny
matmuls -> scan step -> multiply -> store. DMA-bound at ~51 MB/core.

Host-side folding: BN scale/bias folded into w1/w2 (+ the 1/HW mean divisor),
gamma_w replicated to [64,64] so the gate dot lands pre-broadcast on 64
partitions, Wa_b folded in via a K=1 matmul accumulation (skipped when zero).

Engine budget per core (DMA floor ~144us): frame matmuls are pair-batched to
halve PE instruction count; spatial reduces and the output multiplies are
split between DVE and ACT to keep both under the DMA roofline.
"""

import sys

import numpy as np

if "/opt/trn_rl_repo" not in sys.path:
    sys.path.insert(0, "/opt/trn_rl_repo")

_EPS = 1e-5
_NCORES = 8
_B, _T, _C, _H, _W = 16, 8, 512, 28, 28
_HW = _H * _W          # 784
_EPC = _B // _NCORES   # batch elements per core = 2
_F = _EPC * _T         # frames per core = 16
_CH = _C // 128        # channel chunks = 4
_C8 = _C // 8          # gate channels = 64

_CACHE = {}


def _build_program(wab_zero, x_bufs=14, act_reduce_frames=(1, 3, 5)):
    import concourse.bacc as bacc
    import concourse.bass as bass
    import concourse.mybir as mybir
    import concourse.tile as tile

    f32 = mybir.dt.float32
    AF = mybir.ActivationFunctionType
    ALU = mybir.AluOpType

    nc = bacc.Bacc(
        "TRN2",
        target_bir_lowering=False,
        debug=False,
        enable_asserts=False,
        num_devices=_NCORES,
    )

    x_d = nc.dram_tensor("x", [_F, _C, _HW], f32, kind="ExternalInput")
    w1t_d = nc.dram_tensor("w1t", [_C, _C8], f32, kind="ExternalInput")
    w2t_d = nc.dram_tensor("w2t", [_C, _C8], f32, kind="ExternalInput")
    t12_d = nc.dram_tensor("t12", [_C8, 2], f32, kind="ExternalInput")
    gwa_d = nc.dram_tensor("gwa", [_C8, _C8], f32, kind="ExternalInput")
    gwb_d = nc.dram_tensor("gwb", [_C8, _C8], f32, kind="ExternalInput")
    gbr_d = nc.dram_tensor("gbr", [_C8, 1], f32, kind="ExternalInput")
    wat_d = nc.dram_tensor("wat", [_C8, _C], f32, kind="ExternalInput")
    wab_d = nc.dram_tensor("wab", [1, _C], f32, kind="ExternalInput")
    out_d = nc.dram_tensor("out", [_F, _C, _HW], f32, kind="ExternalOutput")

    with tile.TileContext(nc) as tc:
        with (
            tc.tile_pool(name="xp", bufs=x_bufs) as xp,
            tc.tile_pool(name="pers", bufs=1) as pers,
            tc.tile_pool(name="small", bufs=3) as small,
            tc.tile_pool(name="scanp", bufs=2) as scanp,
            tc.tile_pool(name="plr", bufs=2, space=bass.MemorySpace.PSUM) as plr,
            tc.tile_pool(name="pscan", bufs=2, space=bass.MemorySpace.PSUM) as pscan,
        ):
            w1t_s = pers.tile([128, _CH, _C8], f32, tag="w1t")
            w2t_s = pers.tile([128, _CH, _C8], f32, tag="w2t")
            t12_s = pers.tile([_C8, 2], f32, tag="t12")
            gwa_s = pers.tile([_C8, _C8], f32, tag="gwa")
            gwb_s = pers.tile([_C8, _C8], f32, tag="gwb")
            gbr_s = pers.tile([_C8, 1], f32, tag="gbr")
            wat_s = pers.tile([_C8, _C], f32, tag="wat")
            wab_s = pers.tile([1, _C], f32, tag="wab")
            one_s = pers.tile([1, 1], f32, tag="one")
            st0_s = pers.tile([_C8, 1], f32, tag="st0")
            left = pers.tile([_C8, _F], f32, tag="left")
            right = pers.tile([_C8, _F], f32, tag="right")
            diff = pers.tile([_C8, _F], f32, tag="diff")
            sig = pers.tile([128, _CH, _F], f32, tag="sig")

            # Small parameter loads go on the scalar HWDGE ring (idle at start)
            # so they neither delay the first x loads on the sync ring nor the
            # stores on the gpsimd ring.
            nc.scalar.dma_start(w1t_s[:], w1t_d.ap().rearrange("(j p) m -> p j m", p=128))
            nc.scalar.dma_start(w2t_s[:], w2t_d.ap().rearrange("(j p) m -> p j m", p=128))
            nc.scalar.dma_start(t12_s[:], t12_d.ap())
            nc.scalar.dma_start(gwa_s[:], gwa_d.ap())
            nc.scalar.dma_start(gwb_s[:], gwb_d.ap())
            nc.scalar.dma_start(gbr_s[:], gbr_d.ap())
            nc.scalar.dma_start(wat_s[:], wat_d.ap())
            if not wab_zero:
                nc.scalar.dma_start(wab_s[:], wab_d.ap())
            nc.vector.memset(one_s[:], 1.0)
            nc.vector.memset(st0_s[:], 1.0)
            # touch both ACT LUTs once at startup so the first real
            # relu/sigmoid doesn't eat an ACT_TABLE_LOAD on the critical path
            warm = scanp.tile([1, 1], f32, tag="warm")
            nc.scalar.activation(warm[:], one_s[:], AF.Relu)
            nc.scalar.activation(warm[:], one_s[:], AF.Sigmoid)
            for e in range(_EPC):
                # diff at t = T-1 is the constant-1 pad (also the scan init)
                nc.vector.memset(diff[:, (e + 1) * _T - 1 : (e + 1) * _T], 1.0)

            def load_frame(f, rsp, i):
                xt = xp.tile([128, _CH, _HW], f32, tag="x")
                eng = nc.scalar if f % _T == 1 else nc.sync
                eng.dma_start(xt[:], x_d.ap()[f].rearrange("(j p) s -> p j s", p=128))
                if f % _T in act_reduce_frames:
                    for j in range(_CH):
                        nc.scalar.activation(xt[:, j, :], xt[:, j, :], AF.Copy,
                                             accum_out=rsp[:, j, i : i + 1])
                else:
                    nc.vector.reduce_sum(rsp[:, :, i], xt[:], axis=mybir.AxisListType.X)
                return xt

            def lr_matmul(f0, rsp, n):
                # left/right pre-activations for frames [f0, f0+n) in one batch
                pl = plr.tile([_C8, 2], f32, tag="pl")
                pr = plr.tile([_C8, 2], f32, tag="pr")
                for j in range(_CH):
                    nc.tensor.matmul(pl[:, 0:n], w1t_s[:, j, :], rsp[:, j, 0:n],
                                     start=(j == 0), stop=(j == _CH - 1))
                for j in range(_CH):
                    nc.tensor.matmul(pr[:, 0:n], w2t_s[:, j, :], rsp[:, j, 0:n],
                                     start=(j == 0), stop=(j == _CH - 1))
                nc.scalar.activation(left[:, f0 : f0 + n], pl[:, 0:n], AF.Relu,
                                     bias=t12_s[:, 0:1])
                nc.scalar.activation(right[:, f0 : f0 + n], pr[:, 0:n], AF.Relu,
                                     bias=t12_s[:, 1:2])

            def state_step(f, st_prev):
                d = diff[:, f : f + 1]
                pg = pscan.tile([_C8, 1], f32, tag="pg")
                nc.tensor.matmul(pg[:], gwa_s[:], d, start=True, stop=False)
                nc.tensor.matmul(pg[:], gwb_s[:], st_prev[:], start=False, stop=True)
                g = scanp.tile([_C8, 1], f32, tag="g")
                nc.scalar.activation(g[:], pg[:], AF.Sigmoid, bias=gbr_s[:, 0:1])
                tmp = scanp.tile([_C8, 1], f32, tag="tmp")
                nc.vector.tensor_sub(tmp[:], d, st_prev[:])
                st = scanp.tile([_C8, 1], f32, tag="st")
                nc.vector.scalar_tensor_tensor(
                    st[:], tmp[:], g[:], st_prev[:], op0=ALU.mult, op1=ALU.add
                )
                return st

            def att_step(f, st):
                pa = pscan.tile([128, _CH], f32, tag="pa")
                for j in range(_CH):
                    if wab_zero:
                        nc.tensor.matmul(pa[:, j : j + 1], wat_s[:, j * 128 : (j + 1) * 128],
                                         st[:], start=True, stop=True)
                    else:
                        nc.tensor.matmul(pa[:, j : j + 1], wab_s[:, j * 128 : (j + 1) * 128],
                                         one_s[:], start=True, stop=False)
                        nc.tensor.matmul(pa[:, j : j + 1], wat_s[:, j * 128 : (j + 1) * 128],
                                         st[:], start=False, stop=True)
                nc.scalar.activation(sig[:, :, f], pa[:], AF.Sigmoid)

            def scan_step(f, st_prev):
                st = state_step(f, st_prev)
                att_step(f, st)
                return st

            def mul_store(f, xt):
                # split the 4 chunk multiplies across DVE and ACT to halve latency
                for j in range(_CH):
                    if j % 2 == 0:
                        nc.vector.tensor_scalar_mul(xt[:, j, :], xt[:, j, :],
                                                    sig[:, j, f : f + 1])
                    else:
                        nc.scalar.mul(xt[:, j, :], xt[:, j, :], sig[:, j, f : f + 1])
                eng = nc.gpsimd if f % 2 == 0 else nc.scalar
                eng.dma_start(out_d.ap()[f].rearrange("(j p) s -> p j s", p=128), xt[:])

            for e in range(_EPC):
                xts = {}
                st = st0_s
                for k in range(_T // 2 - 1):
                    t0 = 2 * k
                    f0 = e * _T + t0
                    rsp = small.tile([128, _CH, 2], f32, tag="rsp")
                    xts[t0] = load_frame(f0, rsp, 0)
                    xts[t0 + 1] = load_frame(f0 + 1, rsp, 1)
                    lr_matmul(f0, rsp, 2)
                    if k >= 1:
                        nc.vector.tensor_sub(diff[:, f0 - 1 : f0 + 1],
                                             left[:, f0 - 1 : f0 + 1],
                                             right[:, f0 : f0 + 2])
                        st = scan_step(f0 - 1, st)
                        mul_store(f0 - 1, xts.pop(t0 - 1))
                    else:
                        nc.vector.tensor_sub(diff[:, f0 : f0 + 1], left[:, f0 : f0 + 1],
                                             right[:, f0 + 1 : f0 + 2])
                    st = scan_step(f0, st)
                    mul_store(f0, xts.pop(t0))
                # frames T-2, T-1 processed solo so the scan tail starts sooner
                for t in (_T - 2, _T - 1):
                    f = e * _T + t
                    rsp = small.tile([128, _CH, 2], f32, tag="rsp")
                    xts[t] = load_frame(f, rsp, 0)
                    lr_matmul(f, rsp, 1)
                    nc.vector.tensor_sub(diff[:, f - 1 : f], left[:, f - 1 : f],
                                         right[:, f : f + 1])
                    if t < _T - 1:
                        st = scan_step(f - 1, st)
                        mul_store(f - 1, xts.pop(t - 1))
                # state chains for the last two steps back-to-back, then atts
                fl = e * _T + _T - 1
                st_a = state_step(fl - 1, st)
                st = state_step(fl, st_a)
                att_step(fl - 1, st_a)
                att_step(fl, st)
                mul_store(fl - 1, xts.pop(_T - 2))
                mul_store(fl, xts.pop(_T - 1))

    nc.compile()
    return nc


def _get_nc(wab_zero=True):
    key = ("nc", wab_zero)
    if key not in _CACHE:
        _CACHE[key] = _build_program(wab_zero)
    return _CACHE[key]


def _prepare_in_maps(inputs):
    f = np.float32
    x = np.ascontiguousarray(np.asarray(inputs["x"], dtype=f))
    w1 = np.asarray(inputs["w1"], dtype=f)
    w2 = np.asarray(inputs["w2"], dtype=f)
    gamma_w = np.asarray(inputs["gamma_w"], dtype=f)
    gamma_b = np.asarray(inputs["gamma_b"], dtype=f)
    Wa_w = np.asarray(inputs["Wa_w"], dtype=f)
    Wa_b = np.asarray(inputs["Wa_b"], dtype=f)

    s1 = np.asarray(inputs["bn1_g"], dtype=f) / np.sqrt(np.asarray(inputs["bn1_v"], dtype=f) + _EPS)
    t1 = np.asarray(inputs["bn1_b"], dtype=f) - np.asarray(inputs["bn1_m"], dtype=f) * s1
    s2 = np.asarray(inputs["bn2_g"], dtype=f) / np.sqrt(np.asarray(inputs["bn2_v"], dtype=f) + _EPS)
    t2 = np.asarray(inputs["bn2_b"], dtype=f) - np.asarray(inputs["bn2_m"], dtype=f) * s2

    shared = {
        "w1t": np.ascontiguousarray((w1 * s1[:, None] / _HW).T.astype(f)),
        "w2t": np.ascontiguousarray((w2 * s2[:, None] / _HW).T.astype(f)),
        "t12": np.ascontiguousarray(np.stack([t1, t2], axis=1).astype(f)),
        "gwa": np.ascontiguousarray(np.repeat(gamma_w[:_C8, None], _C8, axis=1).astype(f)),
        "gwb": np.ascontiguousarray(np.repeat(gamma_w[_C8:, None], _C8, axis=1).astype(f)),
        "gbr": np.full((_C8, 1), gamma_b[0], dtype=f),
        "wat": np.ascontiguousarray(Wa_w.T.astype(f)),
        "wab": np.ascontiguousarray(Wa_b[None, :].astype(f)),
    }
    in_maps = []
    for c in range(_NCORES):
        m = dict(shared)
        m["x"] = np.ascontiguousarray(
            x[c * _F : (c + 1) * _F].reshape(_F, _C, _HW)
        )
        in_maps.append(m)
    return in_maps, bool(np.all(Wa_b == 0.0))


def _run(inputs, trace=False, **kwargs):
    from concourse.bass_utils import run_bass_kernel_spmd

    assert int(inputs["n_segment"]) == _T
    in_maps, wab_zero = _prepare_in_maps(inputs)
    nc = _get_nc(wab_zero)
    res = run_bass_kernel_spmd(nc, in_maps, list(range(_NCORES)), trace=trace, **kwargs)
    out = np.concatenate([res.results[c]["out"] for c in range(_NCORES)], axis=0)
    return out.reshape(_B * _T, _C, _H, _W), res


def kernel(**inputs) -> np.ndarray:
    out, _ = _run(inputs, trace=False)
    return out


# revision 14
# speedup vs baseline: 1.0607x; 1.0607x over previous
"""Trainium2 Bass/Tile kernel for nn_AccumAtt (temporal accumulated attention).

Pipeline (per (b, t) frame of x [B*T, C, H, W]):
  xv = spatial mean -> left/right = relu(BN(xv @ w{1,2}.T)) -> temporal diff
  -> sequential gate scan over T -> att = sigmoid(new @ Wa.T) -> out = x * att.

Sharding: data-parallel over batch. 8 cores x 2 batch elements each; params
replicated. Single kernel streams each frame once: load -> reduce -> tiny
matmuls -> scan step -> multiply -> store. DMA-bound at ~51 MB/core.

Host-side folding: BN scale/bias folded into w1/w2 (+ the 1/HW mean divisor),
gamma_w replicated to [64,64] so the gate dot lands pre-broadcast on 64
partitions, Wa_b folded in via a K=1 matmul accumulation (skipped when zero).

Engine budget per core (DMA floor ~144us): frame matmuls are pair-batched to
halve PE instruction count; spatial reduces and the output multiplies are
split between DVE and ACT to keep both under the DMA roofline.
"""

import sys

import numpy as np

if "/opt/trn_rl_repo" not in sys.path:
    sys.path.insert(0, "/opt/trn_rl_repo")

_EPS = 1e-5
_NCORES = 8
_B, _T, _C, _H, _W = 16, 8, 512, 28, 28
_HW = _H * _W          # 784
_EPC = _B // _NCORES   # batch elements per core = 2
_F = _EPC * _T         # frames per core = 16
_CH = _C // 128        # channel chunks = 4
_C8 = _C // 8          # gate channels = 64

_CACHE = {}


def _build_program(wab_zero, x_bufs=14, act_reduce_frames=(1, 3, 5)):
    import concourse.bacc as bacc
    import concourse.bass as bass
    import concourse.mybir as mybir
    import concourse.tile as tile

    f32 = mybir.dt.float32
    AF = mybir.ActivationFunctionType
    ALU = mybir.AluOpType

    nc = bacc.Bacc(
        "TRN2",
        target_bir_lowering=False,
        debug=False,
        enable_asserts=False,
        num_devices=_NCORES,
    )

    x_d = nc.dram_tensor("x", [_F, _C, _HW], f32, kind="ExternalInput")
    w1t_d = nc.dram_tensor("w1t", [_C, _C8], f32, kind="ExternalInput")
    w2t_d = nc.dram_tensor("w2t", [_C, _C8], f32, kind="ExternalInput")
    t12_d = nc.dram_tensor("t12", [_C8, 2], f32, kind="ExternalInput")
    gwa_d = nc.dram_tensor("gwa", [_C8, _C8], f32, kind="ExternalInput")
    gwb_d = nc.dram_tensor("gwb", [_C8, _C8], f32, kind="ExternalInput")
    gbr_d = nc.dram_tensor("gbr", [_C8, 1], f32, kind="ExternalInput")
    wat_d = nc.dram_tensor("wat", [_C8, _C], f32, kind="ExternalInput")
    wab_d = nc.dram_tensor("wab", [1, _C], f32, kind="ExternalInput")
    out_d = nc.dram_tensor("out", [_F, _C, _HW], f32, kind="ExternalOutput")

    with tile.TileContext(nc) as tc:
        with (
            tc.tile_pool(name="xp", bufs=x_bufs) as xp,
            tc.tile_pool(name="pers", bufs=1) as pers,
            tc.tile_pool(name="small", bufs=3) as small,
            tc.tile_pool(name="scanp", bufs=2) as scanp,
            tc.tile_pool(name="plr", bufs=2, space=bass.MemorySpace.PSUM) as plr,
            tc.tile_pool(name="pscan", bufs=2, space=bass.MemorySpace.PSUM) as pscan,
        ):
            w1t_s = pers.tile([128, _CH, _C8], f32, tag="w1t")
            w2t_s = pers.tile([128, _CH, _C8], f32, tag="w2t")
            t12_s = pers.tile([_C8, 2], f32, tag="t12")
            gwa_s = pers.tile([_C8, _C8], f32, tag="gwa")
            gwb_s = pers.tile([_C8, _C8], f32, tag="gwb")
            gbr_s = pers.tile([_C8, 1], f32, tag="gbr")
            wat_s = pers.tile([_C8, _C], f32, tag="wat")
            wab_s = pers.tile([1, _C], f32, tag="wab")
            one_s = pers.tile([1, 1], f32, tag="one")
            st0_s = pers.tile([_C8, 1], f32, tag="st0")
            left = pers.tile([_C8, _F], f32, tag="left")
            right = pers.tile([_C8, _F], f32, tag="right")
            diff = pers.tile([_C8, _F], f32, tag="diff")
            sig = pers.tile([128, _CH, _F], f32, tag="sig")

            # Small parameter loads go on the scalar HWDGE ring (idle at start)
            # so they neither delay the first x loads on the sync ring nor the
            # stores on the gpsimd ring.
            nc.scalar.dma_start(w1t_s[:], w1t_d.ap().rearrange("(j p) m -> p j m", p=128))
            nc.scalar.dma_start(w2t_s[:], w2t_d.ap().rearrange("(j p) m -> p j m", p=128))
            nc.scalar.dma_start(t12_s[:], t12_d.ap())
            nc.scalar.dma_start(gwa_s[:], gwa_d.ap())
            nc.scalar.dma_start(gwb_s[:], gwb_d.ap())
            nc.scalar.dma_start(gbr_s[:], gbr_d.ap())
            nc.scalar.dma_start(wat_s[:], wat_d.ap())
            if not wab_zero:
                nc.scalar.dma_start(wab_s[:], wab_d.ap())
            nc.vector.memset(one_s[:], 1.0)
            nc.vector.memset(st0_s[:], 1.0)
            # touch both ACT LUTs once at startup so the first real
            # relu/sigmoid doesn't eat an ACT_TABLE_LOAD on the critical path
            warm = scanp.tile([1, 1], f32, tag="warm")
            nc.scalar.activation(warm[:], one_s[:], AF.Relu)
            nc.scalar.activation(warm[:], one_s[:], AF.Sigmoid)
            for e in range(_EPC):
                # diff at t = T-1 is the constant-1 pad (also the scan init)
                nc.vector.memset(diff[:, (e + 1) * _T - 1 : (e + 1) * _T], 1.0)

            def load_frame(f, rsp, i):
                xt = xp.tile([128, _CH, _HW], f32, tag="x")
                nc.sync.dma_start(xt[:], x_d.ap()[f].rearrange("(j p) s -> p j s", p=128))
                if f % _T in act_reduce_frames:
                    for j in range(_CH):
                        nc.scalar.activation(xt[:, j, :], xt[:, j, :], AF.Copy,
                                             accum_out=rsp[:, j, i : i + 1])
                else:
                    nc.vector.reduce_sum(rsp[:, :, i], xt[:], axis=mybir.AxisListType.X)
                return xt

            def lr_matmul(f0, rsp, n):
                # left/right pre-activations for frames [f0, f0+n) in one batch
                pl = plr.tile([_C8, 2], f32, tag="pl")
                pr = plr.tile([_C8, 2], f32, tag="pr")
                for j in range(_CH):
                    nc.tensor.matmul(pl[:, 0:n], w1t_s[:, j, :], rsp[:, j, 0:n],
                                     start=(j == 0), stop=(j == _CH - 1))
                for j in range(_CH):
                    nc.tensor.matmul(pr[:, 0:n], w2t_s[:, j, :], rsp[:, j, 0:n],
                                     start=(j == 0), stop=(j == _CH - 1))
                nc.scalar.activation(left[:, f0 : f0 + n], pl[:, 0:n], AF.Relu,
                                     bias=t12_s[:, 0:1])
                nc.scalar.activation(right[:, f0 : f0 + n], pr[:, 0:n], AF.Relu,
                                     bias=t12_s[:, 1:2])

            def state_step(f, st_prev):
                d = diff[:, f : f + 1]
                pg = pscan.tile([_C8, 1], f32, tag="pg")
                nc.tensor.matmul(pg[:], gwa_s[:], d, start=True, stop=False)
                nc.tensor.matmul(pg[:], gwb_s[:], st_prev[:], start=False, stop=True)
                g = scanp.tile([_C8, 1], f32, tag="g")
                nc.scalar.activation(g[:], pg[:], AF.Sigmoid, bias=gbr_s[:, 0:1])
                tmp = scanp.tile([_C8, 1], f32, tag="tmp")
                nc.vector.tensor_sub(tmp[:], d, st_prev[:])
                st = scanp.tile([_C8, 1], f32, tag="st")
                nc.vector.scalar_tensor_tensor(
                    st[:], tmp[:], g[:], st_prev[:], op0=ALU.mult, op1=ALU.add
                )
                return st

            def att_step(f, st):
                pa = pscan.tile([128, _CH], f32, tag="pa")
                for j in range(_CH):
                    if wab_zero:
                        nc.tensor.matmul(pa[:, j : j + 1], wat_s[:, j * 128 : (j + 1) * 128],
                                         st[:], start=True, stop=True)
                    else:
                        nc.tensor.matmul(pa[:, j : j + 1], wab_s[:, j * 128 : (j + 1) * 128],
                                         one_s[:], start=True, stop=False)
                        nc.tensor.matmul(pa[:, j : j + 1], wat_s[:, j * 128 : (j + 1) * 128],
                                         st[:], start=False, stop=True)
                nc.scalar.activation(sig[:, :, f], pa[:], AF.Sigmoid)

            def scan_step(f, st_prev):
                st = state_step(f, st_prev)
                att_step(f, st)
                return st

            def mul_store(f, xt):
                # split the 4 chunk multiplies across DVE and ACT to halve latency
                for j in range(_CH):
                    if j % 2 == 0:
                        nc.vector.tensor_scalar_mul(xt[:, j, :], xt[:, j, :],
                                                    sig[:, j, f : f + 1])
                    else:
                        nc.scalar.mul(xt[:, j, :], xt[:, j, :], sig[:, j, f : f + 1])
                nc.gpsimd.dma_start(out_d.ap()[f].rearrange("(j p) s -> p j s", p=128), xt[:])

            for e in range(_EPC):
                xts = {}
                st = st0_s
                for k in range(_T // 2 - 1):
                    t0 = 2 * k
                    f0 = e * _T + t0
                    rsp = small.tile([128, _CH, 2], f32, tag="rsp")
                    xts[t0] = load_frame(f0, rsp, 0)
                    xts[t0 + 1] = load_frame(f0 + 1, rsp, 1)
                    lr_matmul(f0, rsp, 2)
                    if k >= 1:
                        nc.vector.tensor_sub(diff[:, f0 - 1 : f0 + 1],
                                             left[:, f0 - 1 : f0 + 1],
                                             right[:, f0 : f0 + 2])
                        st = scan_step(f0 - 1, st)
                        mul_store(f0 - 1, xts.pop(t0 - 1))
                    else:
                        nc.vector.tensor_sub(diff[:, f0 : f0 + 1], left[:, f0 : f0 + 1],
                                             right[:, f0 + 1 : f0 + 2])
                    st = scan_step(f0, st)
                    mul_store(f0, xts.pop(t0))
                # frames T-2, T-1 processed solo so the scan tail starts sooner
                for t in (_T - 2, _T - 1):
                    f = e * _T + t
                    rsp = small.tile([128, _CH, 2], f32, tag="rsp")
                    xts[t] = load_frame(f, rsp, 0)
                    lr_matmul(f, rsp, 1)
                    nc.vector.tensor_sub(diff[:, f - 1 : f], left[:, f - 1 : f],
                                         right[:, f : f + 1])
                    if t < _T - 1:
                        st = scan_step(f - 1, st)
                        mul_store(f - 1, xts.pop(t - 1))
                # state chains for the last two steps back-to-back, then atts
                fl = e * _T + _T - 1
                st_a = state_step(fl - 1, st)
                st = state_step(fl, st_a)
                att_step(fl - 1, st_a)
                att_step(fl, st)
                mul_store(fl - 1, xts.pop(_T - 2))
                mul_store(fl, xts.pop(_T - 1))

    nc.compile()
    return nc


def _get_nc(wab_zero=True):
    key = ("nc", wab_zero)
    if key not in _CACHE:
        _CACHE[key] = _build_program(wab_zero)
    return _CACHE[key]


def _prepare_in_maps(inputs):
    f = np.float32
    x = np.ascontiguousarray(np.asarray(inputs["x"], dtype=f))
    w1 = np.asarray(inputs["w1"], dtype=f)
    w2 = np.asarray(inputs["w2"], dtype=f)
    gamma_w = np.asarray(inputs["gamma_w"], dtype=f)
    gamma_b = np.asarray(inputs["gamma_b"], dtype=f)
    Wa_w = np.asarray(inputs["Wa_w"], dtype=f)
    Wa_b = np.asarray(inputs["Wa_b"], dtype=f)

    s1 = np.asarray(inputs["bn1_g"], dtype=f) / np.sqrt(np.asarray(inputs["bn1_v"], dtype=f) + _EPS)
    t1 = np.asarray(inputs["bn1_b"], dtype=f) - np.asarray(inputs["bn1_m"], dtype=f) * s1
    s2 = np.asarray(inputs["bn2_g"], dtype=f) / np.sqrt(np.asarray(inputs["bn2_v"], dtype=f) + _EPS)
    t2 = np.asarray(inputs["bn2_b"], dtype=f) - np.asarray(inputs["bn2_m"], dtype=f) * s2

    shared = {
        "w1t": np.ascontiguousarray((w1 * s1[:, None] / _HW).T.astype(f)),
        "w2t": np.ascontiguousarray((w2 * s2[:, None] / _HW).T.astype(f)),
        "t12": np.ascontiguousarray(np.stack([t1, t2], axis=1).astype(f)),
        "gwa": np.ascontiguousarray(np.repeat(gamma_w[:_C8, None], _C8, axis=1).astype(f)),
        "gwb": np.ascontiguousarray(np.repeat(gamma_w[_C8:, None], _C8, axis=1).astype(f)),
        "gbr": np.full((_C8, 1), gamma_b[0], dtype=f),
        "wat": np.ascontiguousarray(Wa_w.T.astype(f)),
        "wab": np.ascontiguousarray(Wa_b[None, :].astype(f)),
    }
    in_maps = []
    for c in range(_NCORES):
        m = dict(shared)
        m["x"] = np.ascontiguousarray(
            x[c * _F : (c + 1) * _F].reshape(_F, _C, _HW)
        )
        in_maps.append(m)
    return in_maps, bool(np.all(Wa_b == 0.0))


def _run(inputs, trace=False, **kwargs):
    from concourse.bass_utils import run_bass_kernel_spmd

    assert int(inputs["n_segment"]) == _T
    in_maps, wab_zero = _prepare_in_maps(inputs)
    nc = _get_nc(wab_zero)
    res = run_bass_kernel_spmd(nc, in_maps, list(range(_NCORES)), trace=trace, **kwargs)
    out = np.concatenate([res.results[c]["out"] for c in range(_NCORES)], axis=0)
    return out.reshape(_B * _T, _C, _H, _W), res


def kernel(**inputs) -> np.ndarray:
    out, _ = _run(inputs, trace=False)
    return out


# revision 26
# speedup vs baseline: 1.2368x; 1.1660x over previous
"""Trainium2 Bass/Tile kernel for nn_AccumAtt (temporal accumulated attention).

Pipeline (per (b, t) frame of x [B*T, C, H, W]):
  xv = spatial mean -> left/right = relu(BN(xv @ w{1,2}.T)) -> temporal diff
  -> sequential gate scan over T -> att = sigmoid(new @ Wa.T) -> out = x * att.

Sharding: data-parallel over batch. 8 cores x 2 batch elements each; params
replicated. Single kernel streams each frame once: load -> reduce -> tiny
matmuls -> scan step -> multiply -> store. DMA-bound at ~51 MB/core.

Host-side folding: BN scale/bias folded into w1/w2 (+ the 1/HW mean divisor),
gamma_w replicated to [64,64] so the gate dot lands pre-broadcast on 64
partitions, Wa_b folded in via a K=1 matmul accumulation (skipped when zero).

Engine budget per core (DMA floor ~122-144us): frame matmuls are pair-batched
to halve PE instruction count; spatial reduces live on DVE and the output
multiplies on ACT so neither queues behind the other (no head-of-line
blocking), with head/tail frames splitting the multiplies across both engines
for latency. Loads ride the sync HWDGE ring, stores the gpsimd SWDGE ring,
parameter loads the scalar ring.
"""

import sys

import numpy as np

if "/opt/trn_rl_repo" not in sys.path:
    sys.path.insert(0, "/opt/trn_rl_repo")

_EPS = 1e-5
_NCORES = 8
_B, _T, _C, _H, _W = 16, 8, 512, 28, 28
_HW = _H * _W          # 784
_EPC = _B // _NCORES   # batch elements per core = 2
_F = _EPC * _T         # frames per core = 16
_CH = _C // 128        # channel chunks = 4
_C8 = _C // 8          # gate channels = 64

_CACHE = {}


_DEFAULT_CFG = dict(
    x_bufs=15,
    act_reduce_frames=(),  # all spatial reduces on DVE: no head-of-line vs muls
    # mul engine plan: head/tail frames split DVE+ACT for latency; mid frames
    # keep DVE free for reduces (no head-of-line blocking)
    mid_mul_plan="act",    # "act" | "pool" | "split"
    scan_eng="vector",     # engine for the tiny scan sub/stt ops
    warmup=True,
    weight_eng="scalar",
)


def _build_program(wab_zero, **cfg_over):
    cfg = dict(_DEFAULT_CFG, **cfg_over)
    x_bufs = cfg["x_bufs"]
    act_reduce_frames = cfg["act_reduce_frames"]
    import concourse.bacc as bacc
    import concourse.bass as bass
    import concourse.mybir as mybir
    import concourse.tile as tile

    f32 = mybir.dt.float32
    AF = mybir.ActivationFunctionType
    ALU = mybir.AluOpType

    nc = bacc.Bacc(
        "TRN2",
        target_bir_lowering=False,
        debug=False,
        enable_asserts=False,
        num_devices=_NCORES,
    )

    x_d = nc.dram_tensor("x", [_F, _C, _HW], f32, kind="ExternalInput")
    w1t_d = nc.dram_tensor("w1t", [_C, _C8], f32, kind="ExternalInput")
    w2t_d = nc.dram_tensor("w2t", [_C, _C8], f32, kind="ExternalInput")
    t12_d = nc.dram_tensor("t12", [_C8, 2], f32, kind="ExternalInput")
    gwa_d = nc.dram_tensor("gwa", [_C8, _C8], f32, kind="ExternalInput")
    gwb_d = nc.dram_tensor("gwb", [_C8, _C8], f32, kind="ExternalInput")
    gbr_d = nc.dram_tensor("gbr", [_C8, 1], f32, kind="ExternalInput")
    wat_d = nc.dram_tensor("wat", [_C8, _C], f32, kind="ExternalInput")
    wab_d = nc.dram_tensor("wab", [1, _C], f32, kind="ExternalInput")
    out_d = nc.dram_tensor("out", [_F, _C, _HW], f32, kind="ExternalOutput")

    with tile.TileContext(nc) as tc:
        with (
            tc.tile_pool(name="xp", bufs=x_bufs) as xp,
            tc.tile_pool(name="pers", bufs=1) as pers,
            tc.tile_pool(name="small", bufs=3) as small,
            tc.tile_pool(name="scanp", bufs=2) as scanp,
            tc.tile_pool(name="plr", bufs=2, space=bass.MemorySpace.PSUM) as plr,
            tc.tile_pool(name="pscan", bufs=2, space=bass.MemorySpace.PSUM) as pscan,
        ):
            w1t_s = pers.tile([128, _CH, _C8], f32, tag="w1t")
            w2t_s = pers.tile([128, _CH, _C8], f32, tag="w2t")
            t12_s = pers.tile([_C8, 2], f32, tag="t12")
            gwa_s = pers.tile([_C8, _C8], f32, tag="gwa")
            gwb_s = pers.tile([_C8, _C8], f32, tag="gwb")
            gbr_s = pers.tile([_C8, 1], f32, tag="gbr")
            wat_s = pers.tile([_C8, _C], f32, tag="wat")
            wab_s = pers.tile([1, _C], f32, tag="wab")
            one_s = pers.tile([1, 1], f32, tag="one")
            st0_s = pers.tile([_C8, 1], f32, tag="st0")
            left = pers.tile([_C8, _F], f32, tag="left")
            right = pers.tile([_C8, _F], f32, tag="right")
            diff = pers.tile([_C8, _F], f32, tag="diff")
            sig = pers.tile([128, _CH, _F], f32, tag="sig")

            # Small parameter loads go on the scalar HWDGE ring (idle at start)
            # so they neither delay the first x loads on the sync ring nor the
            # stores on the gpsimd ring.
            weng = {"scalar": nc.scalar, "gpsimd": nc.gpsimd, "sync": nc.sync}[cfg["weight_eng"]]
            weng.dma_start(w1t_s[:], w1t_d.ap().rearrange("(j p) m -> p j m", p=128))
            weng.dma_start(w2t_s[:], w2t_d.ap().rearrange("(j p) m -> p j m", p=128))
            weng.dma_start(t12_s[:], t12_d.ap())
            weng.dma_start(gwa_s[:], gwa_d.ap())
            weng.dma_start(gwb_s[:], gwb_d.ap())
            weng.dma_start(gbr_s[:], gbr_d.ap())
            weng.dma_start(wat_s[:], wat_d.ap())
            if not wab_zero:
                weng.dma_start(wab_s[:], wab_d.ap())
            nc.vector.memset(one_s[:], 1.0)
            nc.vector.memset(st0_s[:], 1.0)
            if cfg["warmup"]:
                # touch both ACT LUTs once at startup so the first real
                # relu/sigmoid doesn't eat an ACT_TABLE_LOAD mid-kernel
                warm = scanp.tile([1, 1], f32, tag="warm")
                nc.scalar.activation(warm[:], one_s[:], AF.Relu)
                nc.scalar.activation(warm[:], one_s[:], AF.Sigmoid)
            for e in range(_EPC):
                # diff at t = T-1 is the constant-1 pad (also the scan init)
                nc.vector.memset(diff[:, (e + 1) * _T - 1 : (e + 1) * _T], 1.0)

            def load_frame(f, rsp, i):
                xt = xp.tile([128, _CH, _HW], f32, tag="x")
                src = x_d.ap()[f].rearrange("(j p) s -> p j s", p=128)
                if f < 2:
                    # first pair: half-frame loads + partial-sum reduces so the
                    # first scan step (and with it the store stream) starts
                    # ~10us earlier
                    hw2 = _HW // 2
                    nc.sync.dma_start(xt[:, :, 0:hw2], src[:, :, 0:hw2])
                    nc.sync.dma_start(xt[:, :, hw2:], src[:, :, hw2:])
                    rh = small.tile([128, _CH, 2], f32, tag="rhalf")
                    nc.vector.reduce_sum(rh[:, :, 0], xt[:, :, 0:hw2],
                                         axis=mybir.AxisListType.X)
                    nc.vector.reduce_sum(rh[:, :, 1], xt[:, :, hw2:],
                                         axis=mybir.AxisListType.X)
                    nc.vector.tensor_add(rsp[:, :, i], rh[:, :, 0], rh[:, :, 1])
                    return xt
                nc.sync.dma_start(xt[:], src)
                if f % _T in act_reduce_frames:
                    for j in range(_CH):
                        nc.scalar.activation(xt[:, j, :], xt[:, j, :], AF.Copy,
                                             accum_out=rsp[:, j, i : i + 1])
                else:
                    nc.vector.reduce_sum(rsp[:, :, i], xt[:], axis=mybir.AxisListType.X)
                return xt

            def lr_matmul(f0, rsp, n):
                # left/right pre-activations for frames [f0, f0+n) in one batch
                pl = plr.tile([_C8, 2], f32, tag="pl")
                pr = plr.tile([_C8, 2], f32, tag="pr")
                for j in range(_CH):
                    nc.tensor.matmul(pl[:, 0:n], w1t_s[:, j, :], rsp[:, j, 0:n],
                                     start=(j == 0), stop=(j == _CH - 1))
                for j in range(_CH):
                    nc.tensor.matmul(pr[:, 0:n], w2t_s[:, j, :], rsp[:, j, 0:n],
                                     start=(j == 0), stop=(j == _CH - 1))
                nc.scalar.activation(left[:, f0 : f0 + n], pl[:, 0:n], AF.Relu,
                                     bias=t12_s[:, 0:1])
                nc.scalar.activation(right[:, f0 : f0 + n], pr[:, 0:n], AF.Relu,
                                     bias=t12_s[:, 1:2])

            def state_step(f, st_prev):
                d = diff[:, f : f + 1]
                pg = pscan.tile([_C8, 1], f32, tag="pg")
                nc.tensor.matmul(pg[:], gwa_s[:], d, start=True, stop=False)
                nc.tensor.matmul(pg[:], gwb_s[:], st_prev[:], start=False, stop=True)
                g = scanp.tile([_C8, 1], f32, tag="g")
                nc.scalar.activation(g[:], pg[:], AF.Sigmoid, bias=gbr_s[:, 0:1])
                seng = nc.vector if cfg["scan_eng"] == "vector" else nc.gpsimd
                tmp = scanp.tile([_C8, 1], f32, tag="tmp")
                seng.tensor_sub(tmp[:], d, st_prev[:])
                st = scanp.tile([_C8, 1], f32, tag="st")
                seng.scalar_tensor_tensor(
                    st[:], tmp[:], g[:], st_prev[:], op0=ALU.mult, op1=ALU.add
                )
                return st

            def att_step(f, st):
                pa = pscan.tile([128, _CH], f32, tag="pa")
                for j in range(_CH):
                    if wab_zero:
                        nc.tensor.matmul(pa[:, j : j + 1], wat_s[:, j * 128 : (j + 1) * 128],
                                         st[:], start=True, stop=True)
                    else:
                        nc.tensor.matmul(pa[:, j : j + 1], wab_s[:, j * 128 : (j + 1) * 128],
                                         one_s[:], start=True, stop=False)
                        nc.tensor.matmul(pa[:, j : j + 1], wat_s[:, j * 128 : (j + 1) * 128],
                                         st[:], start=False, stop=True)
                nc.scalar.activation(sig[:, :, f], pa[:], AF.Sigmoid)

            def scan_step(f, st_prev):
                st = state_step(f, st_prev)
                att_step(f, st)
                return st

            def mul_store(f, xt):
                plan = "split" if (f < 2 or f >= 10) else cfg["mid_mul_plan"]
                for j in range(_CH):
                    if plan == "split" and j % 2 == 0:
                        nc.vector.tensor_scalar_mul(xt[:, j, :], xt[:, j, :],
                                                    sig[:, j, f : f + 1])
                    elif plan == "pool" and j % 2 == 0:
                        nc.gpsimd.tensor_scalar_mul(xt[:, j, :], xt[:, j, :],
                                                    sig[:, j, f : f + 1])
                    else:
                        nc.scalar.mul(xt[:, j, :], xt[:, j, :], sig[:, j, f : f + 1])
                nc.gpsimd.dma_start(out_d.ap()[f].rearrange("(j p) s -> p j s", p=128), xt[:])

            for e in range(_EPC):
                xts = {}
                st = st0_s
                for k in range(_T // 2 - 1):
                    t0 = 2 * k
                    f0 = e * _T + t0
                    rsp = small.tile([128, _CH, 2], f32, tag="rsp")
                    xts[t0] = load_frame(f0, rsp, 0)
                    xts[t0 + 1] = load_frame(f0 + 1, rsp, 1)
                    lr_matmul(f0, rsp, 2)
                    if k >= 1:
                        nc.vector.tensor_sub(diff[:, f0 - 1 : f0 + 1],
                                             left[:, f0 - 1 : f0 + 1],
                                             right[:, f0 : f0 + 2])
                        st = scan_step(f0 - 1, st)
                        mul_store(f0 - 1, xts.pop(t0 - 1))
                    else:
                        nc.vector.tensor_sub(diff[:, f0 : f0 + 1], left[:, f0 : f0 + 1],
                                             right[:, f0 + 1 : f0 + 2])
                    st = scan_step(f0, st)
                    mul_store(f0, xts.pop(t0))
                # frames T-2, T-1 processed solo so the scan tail starts sooner
                for t in (_T - 2, _T - 1):
                    f = e * _T + t
                    rsp = small.tile([128, _CH, 2], f32, tag="rsp")
                    xts[t] = load_frame(f, rsp, 0)
                    lr_matmul(f, rsp, 1)
                    nc.vector.tensor_sub(diff[:, f - 1 : f], left[:, f - 1 : f],
                                         right[:, f : f + 1])
                    if t < _T - 1:
                        st = scan_step(f - 1, st)
                        mul_store(f - 1, xts.pop(t - 1))
                # state chains for the last two steps back-to-back, then atts
                fl = e * _T + _T - 1
                st_a = state_step(fl - 1, st)
                st = state_step(fl, st_a)
                att_step(fl - 1, st_a)
                att_step(fl, st)
                mul_store(fl - 1, xts.pop(_T - 2))
                mul_store(fl, xts.pop(_T - 1))

    nc.compile()
    return nc


def _get_nc(wab_zero=True):
    key = ("nc", wab_zero)
    if key not in _CACHE:
        _CACHE[key] = _build_program(wab_zero)
    return _CACHE[key]


def _prepare_in_maps(inputs):
    f = np.float32
    x = np.ascontiguousarray(np.asarray(inputs["x"], dtype=f))
    w1 = np.asarray(inputs["w1"], dtype=f)
    w2 = np.asarray(inputs["w2"], dtype=f)
    gamma_w = np.asarray(inputs["gamma_w"], dtype=f)
    gamma_b = np.asarray(inputs["gamma_b"], dtype=f)
    Wa_w = np.asarray(inputs["Wa_w"], dtype=f)
    Wa_b = np.asarray(inputs["Wa_b"], dtype=f)

    s1 = np.asarray(inputs["bn1_g"], dtype=f) / np.sqrt(np.asarray(inputs["bn1_v"], dtype=f) + _EPS)
    t1 = np.asarray(inputs["bn1_b"], dtype=f) - np.asarray(inputs["bn1_m"], dtype=f) * s1
    s2 = np.asarray(inputs["bn2_g"], dtype=f) / np.sqrt(np.asarray(inputs["bn2_v"], dtype=f) + _EPS)
    t2 = np.asarray(inputs["bn2_b"], dtype=f) - np.asarray(inputs["bn2_m"], dtype=f) * s2

    shared = {
        "w1t": np.ascontiguousarray((w1 * s1[:, None] / _HW).T.astype(f)),
        "w2t": np.ascontiguousarray((w2 * s2[:, None] / _HW).T.astype(f)),
        "t12": np.ascontiguousarray(np.stack([t1, t2], axis=1).astype(f)),
        "gwa": np.ascontiguousarray(np.repeat(gamma_w[:_C8, None], _C8, axis=1).astype(f)),
        "gwb": np.ascontiguousarray(np.repeat(gamma_w[_C8:, None], _C8, axis=1).astype(f)),
        "gbr": np.full((_C8, 1), gamma_b[0], dtype=f),
        "wat": np.ascontiguousarray(Wa_w.T.astype(f)),
        "wab": np.ascontiguousarray(Wa_b[None, :].astype(f)),
    }
    in_maps = []
    for c in range(_NCORES):
        m = dict(shared)
        m["x"] = np.ascontiguousarray(
            x[c * _F : (c + 1) * _F].reshape(_F, _C, _HW)
        )
        in_maps.append(m)
    return in_maps, bool(np.all(Wa_b == 0.0))


def _run(inputs, trace=False, **kwargs):
    from concourse.bass_utils import run_bass_kernel_spmd

    assert int(inputs["n_segment"]) == _T
    in_maps, wab_zero = _prepare_in_maps(inputs)
    nc = _get_nc(wab_zero)
    res = run_bass_kernel_spmd(nc, in_maps, list(range(_NCORES)), trace=trace, **kwargs)
    out = np.concatenate([res.results[c]["out"] for c in range(_NCORES)], axis=0)
    return out.reshape(_B * _T, _C, _H, _W), res


def kernel(**inputs) -> np.ndarray:
    out, _ = _run(inputs, trace=False)
    return out
